# revision 11
# baseline (speedup 1.0000x reference)
"""Trainium2 Bass kernel for MixLoRA sparse MoE (8 experts, top-2, shared base MLP).

Sharding: 2D - 4-way over tokens (512 each) x 2-way over the hidden dim H
(2048 each). Host computes the router (logits/top-2/weights) in fp64 and
load-balances tokens into the 4 quarters so each quarter needs only
`slots` (5 or 6) experts; per-slot routing weights ship as inputs.
Each core computes its token-quarter's fc1/expert work over its H-half,
plus a PARTIAL fc2 (W2 and B2 contractions over its H-half); the host sums
the H-pair partials.

Per-core pipeline (feature-major: partitions = feature slice, free = tokens):
  - common fc1 in PSUM once per chunk (2 H-slices per 2-bank PSUM tile).
  - per-expert LoRA deltas chained in place via difference matmuls
    (K=32: +2*B1[e] rows, -2*B1[e-1] rows).
  - a_e = silu(F_e) on ScalarE (one [128, 2T] instr per expert/chunk).
  - abar += cbc_e * a_e on DVE (mult + pair-tree adds).
  - z'_e = A2stack^T a_e (unweighted) via column-packed matmuls;
    z = z' * c post-scaled into one fp8 tile; the B2 contraction is one
    fp8 DoubleRow matmul per out-slice (both stacks as the two k-tiles).
  - out_partial = W2half^T @ abar + B2 lora, emitted per 128-slice.
Chunks are processed in interleaved PAIRS so the in-order PE queue always
has independent work while ACT runs silu (no head-of-line stalls).
"""

import sys, os
sys.path.insert(0, "/opt/trn_rl_repo")

from contextlib import ExitStack

import numpy as np
import ml_dtypes

import concourse.bass as bass
import concourse.tile as tile
from concourse import mybir, bacc
from concourse.bass_utils import run_bass_kernel_spmd

BF = ml_dtypes.bfloat16
F8 = ml_dtypes.float8_e4m3

NCORES = 8
TQ = 4               # token shards
HH = 2               # H shards
D, H, E, R = 1024, 4096, 8, 16
NT = 2048
T = NT // TQ         # tokens per core (512)
HL = H // HH         # H per core (2048)
KD = D // 128        # 8
MH = HL // 128       # 16 local H slices
MD = D // 128        # 8
SC = 2.0
MCHUNK = 2
NCH = MH // MCHUNK   # 8

f32 = mybir.dt.float32
bf16 = mybir.dt.bfloat16
fp8 = mybir.dt.float8e4
DR = mybir.MatmulPerfMode.DoubleRow


def _ap3(v, d1, d2, stride1):
    """[128, *] AP -> [128, d1, d2] with explicit middle stride."""
    return bass.AP(tensor=v.tensor, offset=v.offset,
                   ap=[list(v.ap[0]), [stride1, d1], [1, d2]])


def _build_bass(slots):
    nc = bacc.Bacc("TRN2", target_bir_lowering=False, debug=False)

    xtb = nc.dram_tensor("xtb", [128, KD * T], bf16, kind="ExternalInput")
    w1p = nc.dram_tensor("w1p", [MH, 128, KD * 128], bf16, kind="ExternalInput")
    w2p = nc.dram_tensor("w2p", [MD, 128, MH * 128], bf16, kind="ExternalInput")
    a1s = nc.dram_tensor("a1s", [128, KD * 256], bf16, kind="ExternalInput")
    b1d = nc.dram_tensor("b1d", [2, 128, HL], bf16, kind="ExternalInput")
    a2s = nc.dram_tensor("a2s", [128, MH * 256], bf16, kind="ExternalInput")
    b2q = nc.dram_tensor("b2q", [128, MD * 2 * 128], fp8, kind="ExternalInput")
    cbc = nc.dram_tensor("cbc", [128, slots * T], bf16, kind="ExternalInput")
    cz = nc.dram_tensor("cz", [128, 2 * T], bf16, kind="ExternalInput")
    outt = nc.dram_tensor("outt", [128, MD * T], bf16, kind="ExternalOutput")

    with tile.TileContext(nc) as tc, ExitStack() as ctx:
        consts = ctx.enter_context(tc.tile_pool(name="consts", bufs=1))
        w1cache = ctx.enter_context(tc.tile_pool(name="w1cache", bufs=1))
        w2cache = ctx.enter_context(tc.tile_pool(name="w2cache", bufs=1))
        apool = ctx.enter_context(tc.tile_pool(name="apool", bufs=3))
        cabufs = ctx.enter_context(tc.tile_pool(name="cabufs", bufs=10))
        small = ctx.enter_context(tc.tile_pool(name="small", bufs=2))
        outp = ctx.enter_context(tc.tile_pool(name="outp", bufs=4))
        psF = ctx.enter_context(tc.tile_pool(name="psF", bufs=3, space="PSUM"))
        psZ = ctx.enter_context(tc.tile_pool(name="psZ", bufs=1, space="PSUM"))

        # ---- input loads (issue order = priority; split so first-needed
        # slices land first and many DMA queues run in parallel) ----
        xtb_sb = consts.tile([128, KD * T], bf16, tag="xtb_sb")
        for k in range(KD):
            nc.sync.dma_start(xtb_sb[:, k * T:(k + 1) * T], xtb[:, k * T:(k + 1) * T])
        a1s_sb = consts.tile([128, KD * 256], bf16, tag="a1s_sb")
        nc.sync.dma_start(a1s_sb, a1s[:])
        w1_sb = [w1cache.tile([128, KD * 128], bf16, tag=f"w1_{m}",
                              name=f"w1_sb{m}") for m in range(MH)]
        for m in range(2 * MCHUNK):             # pair-0 W1 slices: hot
            nc.sync.dma_start(w1_sb[m], w1p[m])
        b1d_sb = [consts.tile([128, HL], bf16, tag=f"b1d{s}", name=f"b1d_sb{s}")
                  for s in range(2)]
        for h in range(2):                      # first halves first
            for s in range(2):
                nc.scalar.dma_start(b1d_sb[s][:, h * HL // 2:(h + 1) * HL // 2],
                                    b1d[s][:, h * HL // 2:(h + 1) * HL // 2])
        a2s_sb = consts.tile([128, MH * 256], bf16, tag="a2s_sb")
        for h in range(4):
            nc.scalar.dma_start(a2s_sb[:, h * MH * 64:(h + 1) * MH * 64],
                                a2s[:, h * MH * 64:(h + 1) * MH * 64])
        cbc_sb = consts.tile([128, slots * T], bf16, tag="cbc_sb")
        for e in range(slots):
            nc.scalar.dma_start(cbc_sb[:, e * T:(e + 1) * T],
                                cbc[:, e * T:(e + 1) * T])
        cz_sb = consts.tile([128, 2 * T], bf16, tag="cz_sb")
        nc.scalar.dma_start(cz_sb, cz[:])
        for m in range(2 * MCHUNK, 3 * MCHUNK):  # pair-1 W1
            nc.sync.dma_start(w1_sb[m], w1p[m])
        b2q_sb = consts.tile([128, MD * 2 * 128], fp8, tag="b2q_sb")
        nc.scalar.dma_start(b2q_sb, b2q[:])
        for m in range(3 * MCHUNK, MH):          # remaining W1
            nc.sync.dma_start(w1_sb[m], w1p[m])
        # W2 fully prefetched (needed only for the tail fc2; lowest priority)
        w2_sb = [w2cache.tile([128, MH * 128], bf16, tag=f"w2_{m2}",
                              name=f"w2_sb{m2}") for m2 in range(MD)]
        for m2 in range(MD):
            nc.gpsimd.dma_start(w2_sb[m2], w2p[m2])

        def xtb_k(k):
            return xtb_sb[:, k * T:(k + 1) * T]

        def bcast_mi(v):     # [128, T] -> [128, MCHUNK, T] stride-0 broadcast
            return bass.AP(tensor=v.tensor, offset=v.offset,
                           ap=[list(v.ap[0]), [0, MCHUNK], [1, T]])

        abar = consts.tile([128, MH * T], bf16, tag="abar")
        zps = [psZ.tile([128, T], f32, tag=f"z{s}", name=f"zps{s}") for s in range(2)]

        # ---- chunk-pair pipeline pieces ----
        def emit_fills(ch):
            m0 = ch * MCHUNK
            fmm = psF.tile([128, MCHUNK * T], f32, tag="mm", name="fmm")
            for mi in range(MCHUNK):
                for k in range(KD):
                    nc.tensor.matmul(
                        fmm[:, mi * T:(mi + 1) * T],
                        w1_sb[m0 + mi][:, k * 128:(k + 1) * 128],
                        xtb_k(k), start=(k == 0), stop=False)
            return fmm

        def emit_delta(fmm, ch, e):
            m0 = ch * MCHUNK
            s, g = divmod(e, 4)
            for mi in range(MCHUNK):
                m = m0 + mi
                nc.tensor.matmul(
                    fmm[:, mi * T:(mi + 1) * T],
                    b1d_sb[s][32 * g:32 * g + 32, m * 128:(m + 1) * 128],
                    up_sb[s][32 * g:32 * g + 32, :],
                    start=False, stop=True,
                    skip_group_check=(e > 0),
                    tile_position=(32 * g, 0))

        def emit_silu(fmm, a_ch, e):
            nc.scalar.activation(
                a_ch[:, e * MCHUNK * T:(e + 1) * MCHUNK * T], fmm,
                mybir.ActivationFunctionType.Silu)

        def emit_z(a_ch, ch, e):
            m0 = ch * MCHUNK
            s, j = divmod(e, 4)
            for mi in range(MCHUNK):
                m = m0 + mi
                nc.tensor.matmul(
                    zps[s][32 * j:32 * j + 32, :],
                    a2s_sb[:, m * 256 + s * 128 + 32 * j:m * 256 + s * 128 + 32 * j + 32],
                    a_ch[:, (e * MCHUNK + mi) * T:(e * MCHUNK + mi + 1) * T],
                    start=(m == 0), stop=(m == MH - 1),
                    skip_group_check=True,
                    tile_position=(0, 32 * j))

        def emit_weight_sum(a_ch, ch):
            m0 = ch * MCHUNK
            cas = []
            for e in range(slots):
                ca = cabufs.tile([128, MCHUNK * T], bf16, tag="ca", name=f"ca{e}")
                a_e = a_ch[:, e * MCHUNK * T:(e + 1) * MCHUNK * T]
                nc.vector.tensor_tensor(
                    ca.rearrange("p (mi t) -> p mi t", mi=MCHUNK),
                    a_e.rearrange("p (mi t) -> p mi t", mi=MCHUNK),
                    bcast_mi(cbc_sb[:, e * T:(e + 1) * T]),
                    op=mybir.AluOpType.mult)
                cas.append(ca)
            while len(cas) > 2:
                nxt = []
                for i in range(0, len(cas) - 1, 2):
                    nc.vector.tensor_tensor(cas[i], cas[i], cas[i + 1],
                                            op=mybir.AluOpType.add)
                    nxt.append(cas[i])
                if len(cas) % 2:
                    nxt.append(cas[-1])
                cas = nxt
            ab_sl = abar[:, m0 * T:(m0 + MCHUNK) * T]
            if len(cas) == 2:
                nc.vector.tensor_tensor(ab_sl, cas[0], cas[1],
                                        op=mybir.AluOpType.add)
            else:
                nc.vector.tensor_copy(ab_sl, cas[0])

        # ---- fills for pair 0 go first on the PE queue (smallest DMA deps),
        # u matmuls run while pair-0 expert deps (b1d) land ----
        fmmA = emit_fills(0)
        fmmB = emit_fills(1)

        up_sb = []
        u_ps = psF.tile([128, MCHUNK * T], f32, tag="mm", name="u_ps")
        for s in range(2):
            for k in range(KD):
                nc.tensor.matmul(u_ps[:, s * T:(s + 1) * T],
                                 a1s_sb[:, k * 256 + s * 128:k * 256 + (s + 1) * 128],
                                 xtb_k(k), start=(k == 0), stop=(k == KD - 1))
        for s in range(2):
            u_sb = consts.tile([128, T], bf16, tag=f"u{s}", name=f"u_sb{s}")
            nc.vector.tensor_copy(u_sb, u_ps[:, s * T:(s + 1) * T])
            up_sb.append(u_sb)

        for pair in range(NCH // 2):
            chA, chB = 2 * pair, 2 * pair + 1
            if pair > 0:
                fmmA = emit_fills(chA)
                fmmB = emit_fills(chB)
            a_chA = apool.tile([128, slots * MCHUNK * T], bf16, tag="a", name="a_chA")
            a_chB = apool.tile([128, slots * MCHUNK * T], bf16, tag="a", name="a_chB")
            for e in range(slots):
                emit_delta(fmmA, chA, e)
                emit_delta(fmmB, chB, e)
                emit_silu(fmmA, a_chA, e)
                emit_silu(fmmB, a_chB, e)
                if e > 0:
                    emit_z(a_chA, chA, e - 1)
                    emit_z(a_chB, chB, e - 1)
            emit_z(a_chA, chA, slots - 1)
            emit_z(a_chB, chB, slots - 1)
            emit_weight_sum(a_chA, chA)
            emit_weight_sum(a_chB, chB)

        # ---- z post-scale into one fp8 [128, 2T] tile (s-major) ----
        zq = small.tile([128, 2 * T], fp8, tag="zq")
        for s in range(2):
            na = min(4, max(0, slots - 4 * s))
            if na < 4:
                nc.vector.memset(zq[:, s * T:(s + 1) * T], 0.0)
            if na > 0:
                nc.vector.tensor_tensor(zq[0:32 * na, s * T:s * T + T],
                                        zps[s][0:32 * na, :],
                                        cz_sb[0:32 * na, s * T:s * T + T],
                                        op=mybir.AluOpType.mult)

        # ---- partial fc2 in m2-pairs: W2half^T @ abar + B2 lora (fp8 DR) ----
        for mp in range(MD // 2):
            o_ps = psF.tile([128, MCHUNK * T], f32, tag="mm", name="o_ps")
            for mh in range(2):
                m2 = 2 * mp + mh
                for k2 in range(MH):
                    nc.tensor.matmul(o_ps[:, mh * T:(mh + 1) * T],
                                     w2_sb[m2][:, k2 * 128:(k2 + 1) * 128],
                                     abar[:, k2 * T:(k2 + 1) * T],
                                     start=(k2 == 0), stop=False)
                nc.tensor.matmul(o_ps[:, mh * T:(mh + 1) * T],
                                 _ap3(b2q_sb[:, m2 * 2 * 128:(m2 + 1) * 2 * 128],
                                      2, 128, 128),
                                 _ap3(zq, 2, T, T),
                                 start=False, stop=True, perf_mode=DR)
            for mh in range(2):
                m2 = 2 * mp + mh
                o_sb = outp.tile([128, T], bf16, tag="osb")
                nc.vector.tensor_copy(o_sb, o_ps[:, mh * T:(mh + 1) * T])
                nc.sync.dma_start(outt[:, m2 * T:(m2 + 1) * T], o_sb)

    nc.compile()
    return nc


# ---------------- host side ----------------

def _maxflow_assign(cnt_by_pair, blocks, cap):
    """Exact transportation: pair-class -> eligible quarters, cap per quarter.
    Returns {pair: {q: n}} or None. Dinic on a tiny graph."""
    elig = {}
    for p, n in cnt_by_pair.items():
        i, j = p
        qs = tuple(q for q, S in enumerate(blocks) if i in S and j in S)
        if not qs:
            return None
        elig.setdefault(qs, []).append(p)
    classes = list(elig)
    C, Q = len(classes), len(blocks)
    S, Tk = 0, C + Q + 1
    cap_m = {}
    def add(u, v, c):
        cap_m[(u, v)] = cap_m.get((u, v), 0) + c
        cap_m.setdefault((v, u), 0)
    total = 0
    for ci, k in enumerate(classes):
        n = sum(cnt_by_pair[p] for p in elig[k])
        add(S, 1 + ci, n)
        total += n
        for q in k:
            add(1 + ci, 1 + C + q, n)
    for q in range(Q):
        add(1 + C + q, Tk, cap)
    from collections import deque
    adj = {}
    for (u, v) in cap_m:
        adj.setdefault(u, []).append(v)
    flow_tot = 0
    while True:
        lvl = {S: 0}
        dq = deque([S])
        while dq:
            u = dq.popleft()
            for v in adj.get(u, []):
                if v not in lvl and cap_m[(u, v)] > 0:
                    lvl[v] = lvl[u] + 1
                    dq.append(v)
        if Tk not in lvl:
            break
        it = {u: 0 for u in adj}
        def dfs(u, f):
            if u == Tk:
                return f
            while it[u] < len(adj[u]):
                v = adj[u][it[u]]
                if cap_m[(u, v)] > 0 and lvl.get(v, -1) == lvl[u] + 1:
                    d = dfs(v, min(f, cap_m[(u, v)]))
                    if d > 0:
                        cap_m[(u, v)] -= d
                        cap_m[(v, u)] += d
                        return d
                it[u] += 1
            return 0
        while True:
            f = dfs(S, 1 << 30)
            if f == 0:
                break
            flow_tot += f
    if flow_tot != total:
        return None
    out = {}
    for ci, k in enumerate(classes):
        got = {q: cap_m[(1 + C + q, 1 + ci)] for q in k
               if cap_m[(1 + C + q, 1 + ci)] > 0}
        pairs = elig[k]
        qiter = [(q, n) for q, n in got.items()]
        qi, left = 0, qiter[0][1] if qiter else 0
        for p in pairs:
            need = cnt_by_pair[p]
            out[p] = {}
            while need > 0:
                q, _ = qiter[qi]
                take = min(need, left)
                out[p][q] = out[p].get(q, 0) + take
                need -= take
                left -= take
                if left == 0 and qi + 1 < len(qiter):
                    qi += 1
                    left = qiter[qi][1]
    return out


def _route_and_balance(sel):
    """Host balancing: tokens (with top-2 expert pairs) -> 4 quarters of T
    tokens, each quarter covering its pairs with `slots` experts."""
    pair_of = [tuple(sorted(sel[t])) for t in range(NT)]
    cnt = {}
    toks_by_pair = {}
    for t, p in enumerate(pair_of):
        cnt[p] = cnt.get(p, 0) + 1
        toks_by_pair.setdefault(p, []).append(t)

    import itertools
    rng = np.random.RandomState(0)
    all5 = list(itertools.combinations(range(8), 5))

    def try_blocks(blocks):
        if not all(any(i in S and j in S for S in blocks)
                   for i in range(8) for j in range(i + 1, 8)):
            return None
        return _maxflow_assign(cnt, blocks, T)

    solution = None
    for trial in range(4000):
        idx = rng.choice(len(all5), 4, replace=True)
        blocks = [set(all5[i]) for i in idx]
        r = try_blocks(blocks)
        if r is not None:
            solution = (blocks, r, 5)
            break
    if solution is None:
        all6 = list(itertools.combinations(range(8), 6))
        for trial in range(4000):
            idx = rng.choice(len(all6), 4, replace=True)
            blocks = [set(all6[i]) for i in idx]
            r = try_blocks(blocks)
            if r is not None:
                solution = (blocks, r, 6)
                break
    if solution is None:
        blocks = [set(range(8))] * 4
        solution = (blocks, _maxflow_assign(cnt, blocks, T), 8)

    blocks, assign, slots = solution
    qtoks = [[] for _ in range(TQ)]
    for p, qmap in assign.items():
        toks = toks_by_pair[p]
        i = 0
        for q, n in qmap.items():
            qtoks[q].extend(toks[i:i + n])
            i += n
    perm = np.concatenate([np.array(sorted(qt), dtype=np.int64) for qt in qtoks])
    slot_experts = [sorted(blocks[q]) for q in range(TQ)]
    return perm, slot_experts, slots


def _pack_inputs(hidden_states, gate, W1, b1, W2, b2, A1, B1, A2, B2):
    hs = np.asarray(hidden_states, dtype=np.float64)
    x = hs.reshape(NT, D)
    logits = x @ np.asarray(gate, np.float64).T
    order = np.argsort(-logits, axis=1, kind="stable")
    sel = order[:, :2]                                   # [NT, 2]
    l12 = np.take_along_axis(logits, sel, axis=1)
    w1r = 1.0 / (1.0 + np.exp(-(l12[:, 0] - l12[:, 1])))
    wts = np.stack([w1r, 1.0 - w1r], axis=1)             # [NT, 2]

    perm, slot_experts, slots = _route_and_balance(sel)

    xT = np.ascontiguousarray(x[perm].T.astype(np.float32))    # [D, NT] permuted
    sel_p = sel[perm]
    wts_p = wts[perm]

    W1T = np.asarray(W1, np.float32).T                   # [D, H]
    w1p_full = np.ascontiguousarray(
        W1T.reshape(KD, 128, H // 128, 128).transpose(2, 1, 0, 3)
        .reshape(H // 128, 128, KD * 128)).astype(BF)
    W2T = np.asarray(W2, np.float32).T                   # [H, D]
    w2p_full = np.ascontiguousarray(
        W2T.reshape(H // 128, 128, MD, 128).transpose(2, 1, 0, 3)
        .reshape(MD, 128, (H // 128) * 128)).astype(BF)

    A1 = np.asarray(A1, np.float32)
    B1 = np.asarray(B1, np.float32)
    A2 = np.asarray(A2, np.float32)
    B2 = np.asarray(B2, np.float32)

    assert not np.asarray(b1).any() and not np.asarray(b2).any(), \
        "nonzero biases not supported by this build"

    per_q = []
    for q in range(TQ):
        ex = slot_experts[q]
        S = np.zeros((D, 256), np.float32)
        b1d_full = np.zeros((2, 128, H), np.float32)
        arr = np.zeros((H, 256), np.float32)
        b2sA = np.zeros((2, 128, D), np.float32)
        for si in range(slots):
            s, g = divmod(si, 4)
            base = s * 128 + 32 * g
            S[:, base:base + 16] = A1[ex[si]].T
            b1d_full[s, 32 * g:32 * g + 16, :] = SC * B1[ex[si]].T
            if si > 0:
                S[:, base + 16:base + 32] = A1[ex[si - 1]].T
                b1d_full[s, 32 * g + 16:32 * g + 32, :] = -SC * B1[ex[si - 1]].T
            arr[:, base:base + 16] = A2[ex[si]].T
            b2sA[s, 32 * g:32 * g + 16, :] = SC * B2[ex[si]].T
        a1s = np.ascontiguousarray(
            S.reshape(KD, 128, 256).transpose(1, 0, 2)
            .reshape(128, KD * 256)).astype(BF)
        a2s_full = np.ascontiguousarray(
            arr.reshape(H // 128, 128, 256).transpose(1, 0, 2)
            .reshape(128, (H // 128) * 256)).astype(BF)
        b2qA = np.zeros((128, MD * 2 * 128), np.float32)
        for m2 in range(MD):
            for s in range(2):
                b2qA[:, (m2 * 2 + s) * 128:(m2 * 2 + s + 1) * 128] = \
                    b2sA[s][:, m2 * 128:(m2 + 1) * 128]

        tq_sel = sel_p[q * T:(q + 1) * T]
        tq_wts = wts_p[q * T:(q + 1) * T]
        crow = np.zeros((slots, T), np.float64)
        for si in range(slots):
            m = (tq_sel == ex[si])
            crow[si] = (tq_wts * m).sum(axis=1)
        cbcA = np.ascontiguousarray(
            np.broadcast_to(crow.reshape(1, slots * T), (128, slots * T))
        ).astype(BF)
        czA = np.zeros((2, 128, T), np.float32)
        for si in range(slots):
            s, j = divmod(si, 4)
            czA[s, 32 * j:32 * j + 32, :] = crow[si]
        per_q.append((a1s, b1d_full.astype(BF), a2s_full, b2qA.astype(F8),
                      cbcA, czA.astype(BF)))

    in_maps = []
    for c in range(NCORES):
        tq, hh = divmod(c, HH)
        a1s, b1d_full, a2s_full, b2qA, cbcA, czA = per_q[tq]
        xc = xT[:, tq * T:(tq + 1) * T]
        xcp = np.ascontiguousarray(
            xc.reshape(KD, 128, T).transpose(1, 0, 2).reshape(128, KD * T))
        msl = slice(hh * MH, (hh + 1) * MH)
        in_maps.append({
            "xtb": xcp.astype(BF),
            "w1p": np.ascontiguousarray(w1p_full[msl]),
            "w2p": np.ascontiguousarray(
                w2p_full[:, :, hh * MH * 128:(hh + 1) * MH * 128]),
            "a1s": a1s,
            "b1d": np.ascontiguousarray(b1d_full[:, :, hh * HL:(hh + 1) * HL]),
            "a2s": np.ascontiguousarray(
                a2s_full[:, hh * MH * 256:(hh + 1) * MH * 256]),
            "b2q": b2qA,
            "cbc": cbcA,
            "cz": np.ascontiguousarray(
                czA.transpose(1, 0, 2).reshape(128, 2 * T)),
        })
    return in_maps, perm, slots


_NC_CACHE = {}


def get_nc(slots):
    if slots not in _NC_CACHE:
        _NC_CACHE[slots] = _build_bass(slots)
    return _NC_CACHE[slots]


def _unpack_outputs(results, perm):
    cols = []
    for tq in range(TQ):
        o = None
        for hh in range(HH):
            c = tq * HH + hh
            p = np.asarray(results[c]["outt"], np.float32)
            p = p.reshape(128, MD, T).transpose(1, 0, 2).reshape(D, T)
            o = p if o is None else o + p
        cols.append(o)
    outT = np.concatenate(cols, axis=1)                  # [D, NT] (permuted tokens)
    out = np.empty((NT, D), np.float32)
    out[perm] = outT.T
    return out.reshape(2, NT // 2, D)


def kernel(**inputs):
    in_maps, perm, slots = _pack_inputs(**inputs)
    nc = get_nc(slots)
    res = run_bass_kernel_spmd(nc, in_maps, core_ids=list(range(NCORES)))
    return _unpack_outputs(res.results, perm)


# revision 12
# speedup vs baseline: 1.1259x; 1.1259x over previous
"""Trainium2 Bass kernel for MixLoRA sparse MoE (8 experts, top-2, shared base MLP).

Sharding: 2D - 4-way over tokens (512 each) x 2-way over the hidden dim H
(2048 each). Host computes the router (logits/top-2/weights) in fp64 and
load-balances tokens into the 4 quarters so each quarter needs only
`slots` (5 or 6) experts; per-slot routing weights ship as inputs.
Each core computes its token-quarter's fc1/expert work over its H-half,
plus a PARTIAL fc2 (W2 and B2 contractions over its H-half); the host sums
the H-pair partials.

Per-core pipeline (feature-major: partitions = feature slice, free = tokens):
  - common fc1 in PSUM once per chunk (2 H-slices per 2-bank PSUM tile).
  - per-expert LoRA deltas chained in place via difference matmuls
    (K=32: +2*B1[e] rows, -2*B1[e-1] rows).
  - a_e = silu(F_e) on ScalarE (one [128, 2T] instr per expert/chunk).
  - abar += cbc_e * a_e on DVE (mult + pair-tree adds).
  - z'_e = A2stack^T a_e (unweighted) via column-packed matmuls;
    z = z' * c post-scaled into one fp8 tile; the B2 contraction is one
    fp8 DoubleRow matmul per out-slice (both stacks as the two k-tiles).
  - out_partial = W2half^T @ abar + B2 lora, emitted per 128-slice.
Chunks are processed in interleaved PAIRS so the in-order PE queue always
has independent work while ACT runs silu (no head-of-line stalls).
"""

import sys, os
sys.path.insert(0, "/opt/trn_rl_repo")

from contextlib import ExitStack

import numpy as np
import ml_dtypes

import concourse.bass as bass
import concourse.tile as tile
from concourse import mybir, bacc
from concourse.bass_utils import run_bass_kernel_spmd

BF = ml_dtypes.bfloat16
F8 = ml_dtypes.float8_e4m3

NCORES = 8
TQ = 4               # token shards
HH = 2               # H shards
D, H, E, R = 1024, 4096, 8, 16
NT = 2048
T = NT // TQ         # tokens per core (512)
HL = H // HH         # H per core (2048)
KD = D // 128        # 8
MH = HL // 128       # 16 local H slices
MD = D // 128        # 8
SC = 2.0
MCHUNK = 2
NCH = MH // MCHUNK   # 8

f32 = mybir.dt.float32
bf16 = mybir.dt.bfloat16
fp8 = mybir.dt.float8e4
DR = mybir.MatmulPerfMode.DoubleRow


def _ap3(v, d1, d2, stride1):
    """[128, *] AP -> [128, d1, d2] with explicit middle stride."""
    return bass.AP(tensor=v.tensor, offset=v.offset,
                   ap=[list(v.ap[0]), [stride1, d1], [1, d2]])


def _build_bass(slots):
    nc = bacc.Bacc("TRN2", target_bir_lowering=False, debug=False)

    xtb = nc.dram_tensor("xtb", [128, KD * T], bf16, kind="ExternalInput")
    w1p = nc.dram_tensor("w1p", [MH, 128, KD * 128], bf16, kind="ExternalInput")
    w2p = nc.dram_tensor("w2p", [MD, 128, MH * 128], bf16, kind="ExternalInput")
    a1s = nc.dram_tensor("a1s", [128, KD * 256], bf16, kind="ExternalInput")
    b1d = nc.dram_tensor("b1d", [2, 128, HL], bf16, kind="ExternalInput")
    a2s = nc.dram_tensor("a2s", [128, MH * 256], bf16, kind="ExternalInput")
    b2q = nc.dram_tensor("b2q", [128, MD * 2 * 128], fp8, kind="ExternalInput")
    cbc = nc.dram_tensor("cbc", [128, slots * T], bf16, kind="ExternalInput")
    cz = nc.dram_tensor("cz", [128, 2 * T], bf16, kind="ExternalInput")
    outt = nc.dram_tensor("outt", [128, MD * T], bf16, kind="ExternalOutput")

    with tile.TileContext(nc) as tc, ExitStack() as ctx:
        consts = ctx.enter_context(tc.tile_pool(name="consts", bufs=1))
        w1cache = ctx.enter_context(tc.tile_pool(name="w1cache", bufs=1))
        w2cache = ctx.enter_context(tc.tile_pool(name="w2cache", bufs=1))
        apool = ctx.enter_context(tc.tile_pool(name="apool", bufs=3))
        cabufs = ctx.enter_context(tc.tile_pool(name="cabufs", bufs=10))
        small = ctx.enter_context(tc.tile_pool(name="small", bufs=2))
        outp = ctx.enter_context(tc.tile_pool(name="outp", bufs=4))
        psF = ctx.enter_context(tc.tile_pool(name="psF", bufs=3, space="PSUM"))
        psZ = ctx.enter_context(tc.tile_pool(name="psZ", bufs=1, space="PSUM"))

        # ---- input loads (issue order = priority; split so first-needed
        # slices land first and many DMA queues run in parallel) ----
        xtb_sb = consts.tile([128, KD * T], bf16, tag="xtb_sb")
        for k in range(KD):
            nc.sync.dma_start(xtb_sb[:, k * T:(k + 1) * T], xtb[:, k * T:(k + 1) * T])
        a1s_sb = consts.tile([128, KD * 256], bf16, tag="a1s_sb")
        nc.sync.dma_start(a1s_sb, a1s[:])
        w1_sb = [w1cache.tile([128, KD * 128], bf16, tag=f"w1_{m}",
                              name=f"w1_sb{m}") for m in range(MH)]
        for m in range(2 * MCHUNK):             # pair-0 W1 slices: hot
            nc.sync.dma_start(w1_sb[m], w1p[m])
        b1d_sb = [consts.tile([128, HL], bf16, tag=f"b1d{s}", name=f"b1d_sb{s}")
                  for s in range(2)]
        for h in range(2):                      # first halves first
            for s in range(2):
                nc.sync.dma_start(b1d_sb[s][:, h * HL // 2:(h + 1) * HL // 2],
                                    b1d[s][:, h * HL // 2:(h + 1) * HL // 2])
        a2s_sb = consts.tile([128, MH * 256], bf16, tag="a2s_sb")
        for h in range(4):
            nc.sync.dma_start(a2s_sb[:, h * MH * 64:(h + 1) * MH * 64],
                                a2s[:, h * MH * 64:(h + 1) * MH * 64])
        cbc_sb = consts.tile([128, slots * T], bf16, tag="cbc_sb")
        for e in range(slots):
            nc.sync.dma_start(cbc_sb[:, e * T:(e + 1) * T],
                                cbc[:, e * T:(e + 1) * T])
        cz_sb = consts.tile([128, 2 * T], bf16, tag="cz_sb")
        nc.sync.dma_start(cz_sb, cz[:])
        for m in range(2 * MCHUNK, 3 * MCHUNK):  # pair-1 W1
            nc.sync.dma_start(w1_sb[m], w1p[m])
        b2q_sb = consts.tile([128, MD * 2 * 128], fp8, tag="b2q_sb")
        nc.sync.dma_start(b2q_sb, b2q[:])
        for m in range(3 * MCHUNK, MH):          # remaining W1
            nc.sync.dma_start(w1_sb[m], w1p[m])
        # W2 fully prefetched (needed only for the tail fc2; lowest priority)
        w2_sb = [w2cache.tile([128, MH * 128], bf16, tag=f"w2_{m2}",
                              name=f"w2_sb{m2}") for m2 in range(MD)]
        for m2 in range(MD):
            nc.sync.dma_start(w2_sb[m2], w2p[m2])

        def xtb_k(k):
            return xtb_sb[:, k * T:(k + 1) * T]

        def bcast_mi(v):     # [128, T] -> [128, MCHUNK, T] stride-0 broadcast
            return bass.AP(tensor=v.tensor, offset=v.offset,
                           ap=[list(v.ap[0]), [0, MCHUNK], [1, T]])

        abar = consts.tile([128, MH * T], bf16, tag="abar")
        zps = [psZ.tile([128, T], f32, tag=f"z{s}", name=f"zps{s}") for s in range(2)]

        # ---- chunk-pair pipeline pieces ----
        def emit_fills(ch):
            m0 = ch * MCHUNK
            fmm = psF.tile([128, MCHUNK * T], f32, tag="mm", name="fmm")
            for mi in range(MCHUNK):
                for k in range(KD):
                    nc.tensor.matmul(
                        fmm[:, mi * T:(mi + 1) * T],
                        w1_sb[m0 + mi][:, k * 128:(k + 1) * 128],
                        xtb_k(k), start=(k == 0), stop=False)
            return fmm

        def emit_delta(fmm, ch, e):
            m0 = ch * MCHUNK
            s, g = divmod(e, 4)
            for mi in range(MCHUNK):
                m = m0 + mi
                nc.tensor.matmul(
                    fmm[:, mi * T:(mi + 1) * T],
                    b1d_sb[s][32 * g:32 * g + 32, m * 128:(m + 1) * 128],
                    up_sb[s][32 * g:32 * g + 32, :],
                    start=False, stop=True,
                    skip_group_check=(e > 0),
                    tile_position=(32 * g, 0))

        def emit_silu(fmm, a_ch, e):
            nc.scalar.activation(
                a_ch[:, e * MCHUNK * T:(e + 1) * MCHUNK * T], fmm,
                mybir.ActivationFunctionType.Silu)

        def emit_z(a_ch, ch, e):
            m0 = ch * MCHUNK
            s, j = divmod(e, 4)
            for mi in range(MCHUNK):
                m = m0 + mi
                nc.tensor.matmul(
                    zps[s][32 * j:32 * j + 32, :],
                    a2s_sb[:, m * 256 + s * 128 + 32 * j:m * 256 + s * 128 + 32 * j + 32],
                    a_ch[:, (e * MCHUNK + mi) * T:(e * MCHUNK + mi + 1) * T],
                    start=(m == 0), stop=(m == MH - 1),
                    skip_group_check=True,
                    tile_position=(0, 32 * j))

        def emit_weight_sum(a_ch, ch):
            m0 = ch * MCHUNK
            cas = []
            for e in range(slots):
                ca = cabufs.tile([128, MCHUNK * T], bf16, tag="ca", name=f"ca{e}")
                a_e = a_ch[:, e * MCHUNK * T:(e + 1) * MCHUNK * T]
                nc.vector.tensor_tensor(
                    ca.rearrange("p (mi t) -> p mi t", mi=MCHUNK),
                    a_e.rearrange("p (mi t) -> p mi t", mi=MCHUNK),
                    bcast_mi(cbc_sb[:, e * T:(e + 1) * T]),
                    op=mybir.AluOpType.mult)
                cas.append(ca)
            while len(cas) > 2:
                nxt = []
                for i in range(0, len(cas) - 1, 2):
                    nc.vector.tensor_tensor(cas[i], cas[i], cas[i + 1],
                                            op=mybir.AluOpType.add)
                    nxt.append(cas[i])
                if len(cas) % 2:
                    nxt.append(cas[-1])
                cas = nxt
            ab_sl = abar[:, m0 * T:(m0 + MCHUNK) * T]
            if len(cas) == 2:
                nc.vector.tensor_tensor(ab_sl, cas[0], cas[1],
                                        op=mybir.AluOpType.add)
            else:
                nc.vector.tensor_copy(ab_sl, cas[0])

        # ---- u matmuls first (small DMA deps: a1s + xtb) ----
        up_sb = []
        u_ps = psF.tile([128, MCHUNK * T], f32, tag="mm", name="u_ps")
        for s in range(2):
            for k in range(KD):
                nc.tensor.matmul(u_ps[:, s * T:(s + 1) * T],
                                 a1s_sb[:, k * 256 + s * 128:k * 256 + (s + 1) * 128],
                                 xtb_k(k), start=(k == 0), stop=(k == KD - 1))
        for s in range(2):
            u_sb = consts.tile([128, T], bf16, tag=f"u{s}", name=f"u_sb{s}")
            nc.vector.tensor_copy(u_sb, u_ps[:, s * T:(s + 1) * T])
            up_sb.append(u_sb)

        for pair in range(NCH // 2):
            chA, chB = 2 * pair, 2 * pair + 1
            fmmA = emit_fills(chA)
            fmmB = emit_fills(chB)
            a_chA = apool.tile([128, slots * MCHUNK * T], bf16, tag="a", name="a_chA")
            a_chB = apool.tile([128, slots * MCHUNK * T], bf16, tag="a", name="a_chB")
            for e in range(slots):
                emit_delta(fmmA, chA, e)
                emit_delta(fmmB, chB, e)
                emit_silu(fmmA, a_chA, e)
                emit_silu(fmmB, a_chB, e)
                if e > 0:
                    emit_z(a_chA, chA, e - 1)
                    emit_z(a_chB, chB, e - 1)
            emit_z(a_chA, chA, slots - 1)
            emit_z(a_chB, chB, slots - 1)
            emit_weight_sum(a_chA, chA)
            emit_weight_sum(a_chB, chB)

        # ---- z post-scale into one fp8 [128, 2T] tile (s-major) ----
        zq = small.tile([128, 2 * T], fp8, tag="zq")
        for s in range(2):
            na = min(4, max(0, slots - 4 * s))
            if na < 4:
                nc.vector.memset(zq[:, s * T:(s + 1) * T], 0.0)
            if na > 0:
                nc.vector.tensor_tensor(zq[0:32 * na, s * T:s * T + T],
                                        zps[s][0:32 * na, :],
                                        cz_sb[0:32 * na, s * T:s * T + T],
                                        op=mybir.AluOpType.mult)

        # ---- partial fc2 in m2-pairs: W2half^T @ abar + B2 lora (fp8 DR) ----
        for mp in range(MD // 2):
            o_ps = psF.tile([128, MCHUNK * T], f32, tag="mm", name="o_ps")
            for mh in range(2):
                m2 = 2 * mp + mh
                for k2 in range(MH):
                    nc.tensor.matmul(o_ps[:, mh * T:(mh + 1) * T],
                                     w2_sb[m2][:, k2 * 128:(k2 + 1) * 128],
                                     abar[:, k2 * T:(k2 + 1) * T],
                                     start=(k2 == 0), stop=False)
                nc.tensor.matmul(o_ps[:, mh * T:(mh + 1) * T],
                                 _ap3(b2q_sb[:, m2 * 2 * 128:(m2 + 1) * 2 * 128],
                                      2, 128, 128),
                                 _ap3(zq, 2, T, T),
                                 start=False, stop=True, perf_mode=DR)
            for mh in range(2):
                m2 = 2 * mp + mh
                o_sb = outp.tile([128, T], bf16, tag="osb")
                nc.vector.tensor_copy(o_sb, o_ps[:, mh * T:(mh + 1) * T])
                nc.sync.dma_start(outt[:, m2 * T:(m2 + 1) * T], o_sb)

    nc.compile()
    return nc


# ---------------- host side ----------------

def _maxflow_assign(cnt_by_pair, blocks, cap):
    """Exact transportation: pair-class -> eligible quarters, cap per quarter.
    Returns {pair: {q: n}} or None. Dinic on a tiny graph."""
    elig = {}
    for p, n in cnt_by_pair.items():
        i, j = p
        qs = tuple(q for q, S in enumerate(blocks) if i in S and j in S)
        if not qs:
            return None
        elig.setdefault(qs, []).append(p)
    classes = list(elig)
    C, Q = len(classes), len(blocks)
    S, Tk = 0, C + Q + 1
    cap_m = {}
    def add(u, v, c):
        cap_m[(u, v)] = cap_m.get((u, v), 0) + c
        cap_m.setdefault((v, u), 0)
    total = 0
    for ci, k in enumerate(classes):
        n = sum(cnt_by_pair[p] for p in elig[k])
        add(S, 1 + ci, n)
        total += n
        for q in k:
            add(1 + ci, 1 + C + q, n)
    for q in range(Q):
        add(1 + C + q, Tk, cap)
    from collections import deque
    adj = {}
    for (u, v) in cap_m:
        adj.setdefault(u, []).append(v)
    flow_tot = 0
    while True:
        lvl = {S: 0}
        dq = deque([S])
        while dq:
            u = dq.popleft()
            for v in adj.get(u, []):
                if v not in lvl and cap_m[(u, v)] > 0:
                    lvl[v] = lvl[u] + 1
                    dq.append(v)
        if Tk not in lvl:
            break
        it = {u: 0 for u in adj}
        def dfs(u, f):
            if u == Tk:
                return f
            while it[u] < len(adj[u]):
                v = adj[u][it[u]]
                if cap_m[(u, v)] > 0 and lvl.get(v, -1) == lvl[u] + 1:
                    d = dfs(v, min(f, cap_m[(u, v)]))
                    if d > 0:
                        cap_m[(u, v)] -= d
                        cap_m[(v, u)] += d
                        return d
                it[u] += 1
            return 0
        while True:
            f = dfs(S, 1 << 30)
            if f == 0:
                break
            flow_tot += f
    if flow_tot != total:
        return None
    out = {}
    for ci, k in enumerate(classes):
        got = {q: cap_m[(1 + C + q, 1 + ci)] for q in k
               if cap_m[(1 + C + q, 1 + ci)] > 0}
        pairs = elig[k]
        qiter = [(q, n) for q, n in got.items()]
        qi, left = 0, qiter[0][1] if qiter else 0
        for p in pairs:
            need = cnt_by_pair[p]
            out[p] = {}
            while need > 0:
                q, _ = qiter[qi]
                take = min(need, left)
                out[p][q] = out[p].get(q, 0) + take
                need -= take
                left -= take
                if left == 0 and qi + 1 < len(qiter):
                    qi += 1
                    left = qiter[qi][1]
    return out


def _route_and_balance(sel):
    """Host balancing: tokens (with top-2 expert pairs) -> 4 quarters of T
    tokens, each quarter covering its pairs with `slots` experts."""
    pair_of = [tuple(sorted(sel[t])) for t in range(NT)]
    cnt = {}
    toks_by_pair = {}
    for t, p in enumerate(pair_of):
        cnt[p] = cnt.get(p, 0) + 1
        toks_by_pair.setdefault(p, []).append(t)

    import itertools
    rng = np.random.RandomState(0)
    all5 = list(itertools.combinations(range(8), 5))

    def try_blocks(blocks):
        if not all(any(i in S and j in S for S in blocks)
                   for i in range(8) for j in range(i + 1, 8)):
            return None
        return _maxflow_assign(cnt, blocks, T)

    solution = None
    for trial in range(4000):
        idx = rng.choice(len(all5), 4, replace=True)
        blocks = [set(all5[i]) for i in idx]
        r = try_blocks(blocks)
        if r is not None:
            solution = (blocks, r, 5)
            break
    if solution is None:
        all6 = list(itertools.combinations(range(8), 6))
        for trial in range(4000):
            idx = rng.choice(len(all6), 4, replace=True)
            blocks = [set(all6[i]) for i in idx]
            r = try_blocks(blocks)
            if r is not None:
                solution = (blocks, r, 6)
                break
    if solution is None:
        blocks = [set(range(8))] * 4
        solution = (blocks, _maxflow_assign(cnt, blocks, T), 8)

    blocks, assign, slots = solution
    qtoks = [[] for _ in range(TQ)]
    for p, qmap in assign.items():
        toks = toks_by_pair[p]
        i = 0
        for q, n in qmap.items():
            qtoks[q].extend(toks[i:i + n])
            i += n
    perm = np.concatenate([np.array(sorted(qt), dtype=np.int64) for qt in qtoks])
    slot_experts = [sorted(blocks[q]) for q in range(TQ)]
    return perm, slot_experts, slots


def _pack_inputs(hidden_states, gate, W1, b1, W2, b2, A1, B1, A2, B2):
    hs = np.asarray(hidden_states, dtype=np.float64)
    x = hs.reshape(NT, D)
    logits = x @ np.asarray(gate, np.float64).T
    order = np.argsort(-logits, axis=1, kind="stable")
    sel = order[:, :2]                                   # [NT, 2]
    l12 = np.take_along_axis(logits, sel, axis=1)
    w1r = 1.0 / (1.0 + np.exp(-(l12[:, 0] - l12[:, 1])))
    wts = np.stack([w1r, 1.0 - w1r], axis=1)             # [NT, 2]

    perm, slot_experts, slots = _route_and_balance(sel)

    xT = np.ascontiguousarray(x[perm].T.astype(np.float32))    # [D, NT] permuted
    sel_p = sel[perm]
    wts_p = wts[perm]

    W1T = np.asarray(W1, np.float32).T                   # [D, H]
    w1p_full = np.ascontiguousarray(
        W1T.reshape(KD, 128, H // 128, 128).transpose(2, 1, 0, 3)
        .reshape(H // 128, 128, KD * 128)).astype(BF)
    W2T = np.asarray(W2, np.float32).T                   # [H, D]
    w2p_full = np.ascontiguousarray(
        W2T.reshape(H // 128, 128, MD, 128).transpose(2, 1, 0, 3)
        .reshape(MD, 128, (H // 128) * 128)).astype(BF)

    A1 = np.asarray(A1, np.float32)
    B1 = np.asarray(B1, np.float32)
    A2 = np.asarray(A2, np.float32)
    B2 = np.asarray(B2, np.float32)

    assert not np.asarray(b1).any() and not np.asarray(b2).any(), \
        "nonzero biases not supported by this build"

    per_q = []
    for q in range(TQ):
        ex = slot_experts[q]
        S = np.zeros((D, 256), np.float32)
        b1d_full = np.zeros((2, 128, H), np.float32)
        arr = np.zeros((H, 256), np.float32)
        b2sA = np.zeros((2, 128, D), np.float32)
        for si in range(slots):
            s, g = divmod(si, 4)
            base = s * 128 + 32 * g
            S[:, base:base + 16] = A1[ex[si]].T
            b1d_full[s, 32 * g:32 * g + 16, :] = SC * B1[ex[si]].T
            if si > 0:
                S[:, base + 16:base + 32] = A1[ex[si - 1]].T
                b1d_full[s, 32 * g + 16:32 * g + 32, :] = -SC * B1[ex[si - 1]].T
            arr[:, base:base + 16] = A2[ex[si]].T
            b2sA[s, 32 * g:32 * g + 16, :] = SC * B2[ex[si]].T
        a1s = np.ascontiguousarray(
            S.reshape(KD, 128, 256).transpose(1, 0, 2)
            .reshape(128, KD * 256)).astype(BF)
        a2s_full = np.ascontiguousarray(
            arr.reshape(H // 128, 128, 256).transpose(1, 0, 2)
            .reshape(128, (H // 128) * 256)).astype(BF)
        b2qA = np.zeros((128, MD * 2 * 128), np.float32)
        for m2 in range(MD):
            for s in range(2):
                b2qA[:, (m2 * 2 + s) * 128:(m2 * 2 + s + 1) * 128] = \
                    b2sA[s][:, m2 * 128:(m2 + 1) * 128]

        tq_sel = sel_p[q * T:(q + 1) * T]
        tq_wts = wts_p[q * T:(q + 1) * T]
        crow = np.zeros((slots, T), np.float64)
        for si in range(slots):
            m = (tq_sel == ex[si])
            crow[si] = (tq_wts * m).sum(axis=1)
        cbcA = np.ascontiguousarray(
            np.broadcast_to(crow.reshape(1, slots * T), (128, slots * T))
        ).astype(BF)
        czA = np.zeros((2, 128, T), np.float32)
        for si in range(slots):
            s, j = divmod(si, 4)
            czA[s, 32 * j:32 * j + 32, :] = crow[si]
        per_q.append((a1s, b1d_full.astype(BF), a2s_full, b2qA.astype(F8),
                      cbcA, czA.astype(BF)))

    in_maps = []
    for c in range(NCORES):
        tq, hh = divmod(c, HH)
        a1s, b1d_full, a2s_full, b2qA, cbcA, czA = per_q[tq]
        xc = xT[:, tq * T:(tq + 1) * T]
        xcp = np.ascontiguousarray(
            xc.reshape(KD, 128, T).transpose(1, 0, 2).reshape(128, KD * T))
        msl = slice(hh * MH, (hh + 1) * MH)
        in_maps.append({
            "xtb": xcp.astype(BF),
            "w1p": np.ascontiguousarray(w1p_full[msl]),
            "w2p": np.ascontiguousarray(
                w2p_full[:, :, hh * MH * 128:(hh + 1) * MH * 128]),
            "a1s": a1s,
            "b1d": np.ascontiguousarray(b1d_full[:, :, hh * HL:(hh + 1) * HL]),
            "a2s": np.ascontiguousarray(
                a2s_full[:, hh * MH * 256:(hh + 1) * MH * 256]),
            "b2q": b2qA,
            "cbc": cbcA,
            "cz": np.ascontiguousarray(
                czA.transpose(1, 0, 2).reshape(128, 2 * T)),
        })
    return in_maps, perm, slots


_NC_CACHE = {}


def get_nc(slots):
    if slots not in _NC_CACHE:
        _NC_CACHE[slots] = _build_bass(slots)
    return _NC_CACHE[slots]


def _unpack_outputs(results, perm):
    cols = []
    for tq in range(TQ):
        o = None
        for hh in range(HH):
            c = tq * HH + hh
            p = np.asarray(results[c]["outt"], np.float32)
            p = p.reshape(128, MD, T).transpose(1, 0, 2).reshape(D, T)
            o = p if o is None else o + p
        cols.append(o)
    outT = np.concatenate(cols, axis=1)                  # [D, NT] (permuted tokens)
    out = np.empty((NT, D), np.float32)
    out[perm] = outT.T
    return out.reshape(2, NT // 2, D)


def kernel(**inputs):
    in_maps, perm, slots = _pack_inputs(**inputs)
    nc = get_nc(slots)
    res = run_bass_kernel_spmd(nc, in_maps, core_ids=list(range(NCORES)))
    return _unpack_outputs(res.results, perm)


# revision 13
# speedup vs baseline: 1.1567x; 1.0274x over previous
"""Trainium2 Bass kernel for MixLoRA sparse MoE (8 experts, top-2, shared base MLP).

Sharding: 2D - 4-way over tokens (512 each) x 2-way over the hidden dim H
(2048 each). Host computes the router (logits/top-2/weights) in fp64 and
load-balances tokens into the 4 quarters so each quarter needs only
`slots` (5 or 6) experts; per-slot routing weights ship as inputs.
Each core computes its token-quarter's fc1/expert work over its H-half,
plus a PARTIAL fc2 (W2 and B2 contractions over its H-half); the host sums
the H-pair partials.

Per-core pipeline (feature-major: partitions = feature slice, free = tokens):
  - common fc1 in PSUM once per chunk (2 H-slices per 2-bank PSUM tile).
  - per-expert LoRA deltas chained in place via difference matmuls
    (K=32: +2*B1[e] rows, -2*B1[e-1] rows).
  - a_e = silu(F_e) on ScalarE (one [128, 2T] instr per expert/chunk).
  - abar += cbc_e * a_e on DVE (mult + pair-tree adds).
  - z'_e = A2stack^T a_e (unweighted) via column-packed matmuls;
    z = z' * c post-scaled into one fp8 tile; the B2 contraction is one
    fp8 DoubleRow matmul per out-slice (both stacks as the two k-tiles).
  - out_partial = W2half^T @ abar + B2 lora, emitted per 128-slice.
Chunks are processed in interleaved PAIRS so the in-order PE queue always
has independent work while ACT runs silu (no head-of-line stalls).
"""

import sys, os
sys.path.insert(0, "/opt/trn_rl_repo")

from contextlib import ExitStack

import numpy as np
import ml_dtypes

import concourse.bass as bass
import concourse.tile as tile
from concourse import mybir, bacc
from concourse.bass_utils import run_bass_kernel_spmd

BF = ml_dtypes.bfloat16
F8 = ml_dtypes.float8_e4m3

NCORES = 8
TQ = 4               # token shards
HH = 2               # H shards
D, H, E, R = 1024, 4096, 8, 16
NT = 2048
T = NT // TQ         # tokens per core (512)
HL = H // HH         # H per core (2048)
KD = D // 128        # 8
MH = HL // 128       # 16 local H slices
MD = D // 128        # 8
SC = 2.0
MCHUNK = 2
NCH = MH // MCHUNK   # 8

f32 = mybir.dt.float32
bf16 = mybir.dt.bfloat16
fp8 = mybir.dt.float8e4
DR = mybir.MatmulPerfMode.DoubleRow


def _ap3(v, d1, d2, stride1):
    """[128, *] AP -> [128, d1, d2] with explicit middle stride."""
    return bass.AP(tensor=v.tensor, offset=v.offset,
                   ap=[list(v.ap[0]), [stride1, d1], [1, d2]])


def _build_bass(slots):
    nc = bacc.Bacc("TRN2", target_bir_lowering=False, debug=False)

    xtb = nc.dram_tensor("xtb", [128, KD * T], bf16, kind="ExternalInput")
    w1p = nc.dram_tensor("w1p", [MH, 128, KD * 128], bf16, kind="ExternalInput")
    w2p = nc.dram_tensor("w2p", [MD, 128, MH * 128], bf16, kind="ExternalInput")
    a1s = nc.dram_tensor("a1s", [128, KD * 256], bf16, kind="ExternalInput")
    b1d = nc.dram_tensor("b1d", [2, 128, HL], bf16, kind="ExternalInput")
    a2s = nc.dram_tensor("a2s", [128, MH * 256], bf16, kind="ExternalInput")
    b2q = nc.dram_tensor("b2q", [128, MD * 2 * 128], fp8, kind="ExternalInput")
    cbc = nc.dram_tensor("cbc", [128, slots * T], bf16, kind="ExternalInput")
    cz = nc.dram_tensor("cz", [128, 2 * T], bf16, kind="ExternalInput")
    outt = nc.dram_tensor("outt", [128, MD * T], bf16, kind="ExternalOutput")

    with tile.TileContext(nc) as tc, ExitStack() as ctx:
        consts = ctx.enter_context(tc.tile_pool(name="consts", bufs=1))
        w1cache = ctx.enter_context(tc.tile_pool(name="w1cache", bufs=1))
        w2cache = ctx.enter_context(tc.tile_pool(name="w2cache", bufs=1))
        apool = ctx.enter_context(tc.tile_pool(name="apool", bufs=3))
        cabufs = ctx.enter_context(tc.tile_pool(name="cabufs", bufs=10))
        small = ctx.enter_context(tc.tile_pool(name="small", bufs=2))
        outp = ctx.enter_context(tc.tile_pool(name="outp", bufs=4))
        psF = ctx.enter_context(tc.tile_pool(name="psF", bufs=3, space="PSUM"))
        psZ = ctx.enter_context(tc.tile_pool(name="psZ", bufs=1, space="PSUM"))

        # ---- input loads (issue order = priority; split so first-needed
        # slices land first and many DMA queues run in parallel) ----
        a1s_sb = consts.tile([128, KD * 256], bf16, tag="a1s_sb")
        nc.sync.dma_start(a1s_sb[:, :KD * 128], a1s[:, :KD * 128])
        nc.sync.dma_start(a1s_sb[:, KD * 128:], a1s[:, KD * 128:])
        xtb_sb = consts.tile([128, KD * T], bf16, tag="xtb_sb")
        for k in range(KD):
            nc.sync.dma_start(xtb_sb[:, k * T:(k + 1) * T], xtb[:, k * T:(k + 1) * T])
        w1_sb = [w1cache.tile([128, KD * 128], bf16, tag=f"w1_{m}",
                              name=f"w1_sb{m}") for m in range(MH)]
        for m in range(2 * MCHUNK):             # pair-0 W1 slices: hot
            nc.sync.dma_start(w1_sb[m], w1p[m])
        b1d_sb = [consts.tile([128, HL], bf16, tag=f"b1d{s}", name=f"b1d_sb{s}")
                  for s in range(2)]
        for h in range(2):                      # first halves first
            for s in range(2):
                nc.sync.dma_start(b1d_sb[s][:, h * HL // 2:(h + 1) * HL // 2],
                                    b1d[s][:, h * HL // 2:(h + 1) * HL // 2])
        a2s_sb = consts.tile([128, MH * 256], bf16, tag="a2s_sb")
        for h in range(4):
            nc.sync.dma_start(a2s_sb[:, h * MH * 64:(h + 1) * MH * 64],
                                a2s[:, h * MH * 64:(h + 1) * MH * 64])
        cbc_sb = consts.tile([128, slots * T], bf16, tag="cbc_sb")
        for e in range(slots):
            nc.sync.dma_start(cbc_sb[:, e * T:(e + 1) * T],
                                cbc[:, e * T:(e + 1) * T])
        cz_sb = consts.tile([128, 2 * T], bf16, tag="cz_sb")
        nc.sync.dma_start(cz_sb, cz[:])
        for m in range(2 * MCHUNK, 3 * MCHUNK):  # pair-1 W1
            nc.sync.dma_start(w1_sb[m], w1p[m])
        b2q_sb = consts.tile([128, MD * 2 * 128], fp8, tag="b2q_sb")
        nc.sync.dma_start(b2q_sb, b2q[:])
        for m in range(3 * MCHUNK, MH):          # remaining W1
            nc.sync.dma_start(w1_sb[m], w1p[m])
        # W2 fully prefetched (needed only for the tail fc2; lowest priority)
        w2_sb = [w2cache.tile([128, MH * 128], bf16, tag=f"w2_{m2}",
                              name=f"w2_sb{m2}") for m2 in range(MD)]
        for m2 in range(MD):
            nc.sync.dma_start(w2_sb[m2], w2p[m2])

        def xtb_k(k):
            return xtb_sb[:, k * T:(k + 1) * T]

        def bcast_mi(v):     # [128, T] -> [128, MCHUNK, T] stride-0 broadcast
            return bass.AP(tensor=v.tensor, offset=v.offset,
                           ap=[list(v.ap[0]), [0, MCHUNK], [1, T]])

        abar = consts.tile([128, MH * T], bf16, tag="abar")
        zps = [psZ.tile([128, T], f32, tag=f"z{s}", name=f"zps{s}") for s in range(2)]

        # ---- chunk-pair pipeline pieces ----
        def emit_fills(ch):
            m0 = ch * MCHUNK
            fmm = psF.tile([128, MCHUNK * T], f32, tag="mm", name="fmm")
            for mi in range(MCHUNK):
                for k in range(KD):
                    nc.tensor.matmul(
                        fmm[:, mi * T:(mi + 1) * T],
                        w1_sb[m0 + mi][:, k * 128:(k + 1) * 128],
                        xtb_k(k), start=(k == 0), stop=False)
            return fmm

        def emit_delta(fmm, ch, e):
            m0 = ch * MCHUNK
            s, g = divmod(e, 4)
            for mi in range(MCHUNK):
                m = m0 + mi
                nc.tensor.matmul(
                    fmm[:, mi * T:(mi + 1) * T],
                    b1d_sb[s][32 * g:32 * g + 32, m * 128:(m + 1) * 128],
                    up_sb[s][32 * g:32 * g + 32, :],
                    start=False, stop=True,
                    skip_group_check=(e > 0),
                    tile_position=(32 * g, 0))

        def emit_silu(fmm, a_ch, e):
            nc.scalar.activation(
                a_ch[:, e * MCHUNK * T:(e + 1) * MCHUNK * T], fmm,
                mybir.ActivationFunctionType.Silu)

        def emit_z(a_ch, ch, e):
            m0 = ch * MCHUNK
            s, j = divmod(e, 4)
            for mi in range(MCHUNK):
                m = m0 + mi
                nc.tensor.matmul(
                    zps[s][32 * j:32 * j + 32, :],
                    a2s_sb[:, m * 256 + s * 128 + 32 * j:m * 256 + s * 128 + 32 * j + 32],
                    a_ch[:, (e * MCHUNK + mi) * T:(e * MCHUNK + mi + 1) * T],
                    start=(m == 0), stop=(m == MH - 1),
                    skip_group_check=True,
                    tile_position=(0, 32 * j))

        def emit_weight_sum(a_ch, ch):
            m0 = ch * MCHUNK
            cas = []
            for e in range(slots):
                ca = cabufs.tile([128, MCHUNK * T], bf16, tag="ca", name=f"ca{e}")
                a_e = a_ch[:, e * MCHUNK * T:(e + 1) * MCHUNK * T]
                nc.vector.tensor_tensor(
                    ca.rearrange("p (mi t) -> p mi t", mi=MCHUNK),
                    a_e.rearrange("p (mi t) -> p mi t", mi=MCHUNK),
                    bcast_mi(cbc_sb[:, e * T:(e + 1) * T]),
                    op=mybir.AluOpType.mult)
                cas.append(ca)
            while len(cas) > 2:
                nxt = []
                for i in range(0, len(cas) - 1, 2):
                    nc.vector.tensor_tensor(cas[i], cas[i], cas[i + 1],
                                            op=mybir.AluOpType.add)
                    nxt.append(cas[i])
                if len(cas) % 2:
                    nxt.append(cas[-1])
                cas = nxt
            ab_sl = abar[:, m0 * T:(m0 + MCHUNK) * T]
            if len(cas) == 2:
                nc.vector.tensor_tensor(ab_sl, cas[0], cas[1],
                                        op=mybir.AluOpType.add)
            else:
                nc.vector.tensor_copy(ab_sl, cas[0])

        # ---- u matmuls first (small DMA deps: a1s + xtb) ----
        up_sb = []
        u_ps = psF.tile([128, MCHUNK * T], f32, tag="mm", name="u_ps")
        for s in range(2):
            for k in range(KD):
                nc.tensor.matmul(u_ps[:, s * T:(s + 1) * T],
                                 a1s_sb[:, k * 256 + s * 128:k * 256 + (s + 1) * 128],
                                 xtb_k(k), start=(k == 0), stop=(k == KD - 1))
        for s in range(2):
            u_sb = consts.tile([128, T], bf16, tag=f"u{s}", name=f"u_sb{s}")
            nc.vector.tensor_copy(u_sb, u_ps[:, s * T:(s + 1) * T])
            up_sb.append(u_sb)

        for pair in range(NCH // 2):
            chA, chB = 2 * pair, 2 * pair + 1
            fmmA = emit_fills(chA)
            fmmB = emit_fills(chB)
            a_chA = apool.tile([128, slots * MCHUNK * T], bf16, tag="a", name="a_chA")
            a_chB = apool.tile([128, slots * MCHUNK * T], bf16, tag="a", name="a_chB")
            for e in range(slots):
                emit_delta(fmmA, chA, e)
                emit_delta(fmmB, chB, e)
                emit_silu(fmmA, a_chA, e)
                emit_silu(fmmB, a_chB, e)
                if e > 0:
                    emit_z(a_chA, chA, e - 1)
                    emit_z(a_chB, chB, e - 1)
            emit_z(a_chA, chA, slots - 1)
            emit_z(a_chB, chB, slots - 1)
            emit_weight_sum(a_chA, chA)
            emit_weight_sum(a_chB, chB)

        # ---- z post-scale into one fp8 [128, 2T] tile (s-major) ----
        zq = small.tile([128, 2 * T], fp8, tag="zq")
        for s in range(2):
            na = min(4, max(0, slots - 4 * s))
            if na < 4:
                nc.vector.memset(zq[:, s * T:(s + 1) * T], 0.0)
            if na > 0:
                nc.vector.tensor_tensor(zq[0:32 * na, s * T:s * T + T],
                                        zps[s][0:32 * na, :],
                                        cz_sb[0:32 * na, s * T:s * T + T],
                                        op=mybir.AluOpType.mult)

        # ---- partial fc2 in m2-pairs: W2half^T @ abar + B2 lora (fp8 DR) ----
        for mp in range(MD // 2):
            o_ps = psF.tile([128, MCHUNK * T], f32, tag="mm", name="o_ps")
            for mh in range(2):
                m2 = 2 * mp + mh
                for k2 in range(MH):
                    nc.tensor.matmul(o_ps[:, mh * T:(mh + 1) * T],
                                     w2_sb[m2][:, k2 * 128:(k2 + 1) * 128],
                                     abar[:, k2 * T:(k2 + 1) * T],
                                     start=(k2 == 0), stop=False)
                nc.tensor.matmul(o_ps[:, mh * T:(mh + 1) * T],
                                 _ap3(b2q_sb[:, m2 * 2 * 128:(m2 + 1) * 2 * 128],
                                      2, 128, 128),
                                 _ap3(zq, 2, T, T),
                                 start=False, stop=True, perf_mode=DR)
            for mh in range(2):
                m2 = 2 * mp + mh
                o_sb = outp.tile([128, T], bf16, tag="osb")
                nc.vector.tensor_copy(o_sb, o_ps[:, mh * T:(mh + 1) * T])
                nc.sync.dma_start(outt[:, m2 * T:(m2 + 1) * T], o_sb)

    nc.compile()
    return nc


# ---------------- host side ----------------

def _maxflow_assign(cnt_by_pair, blocks, cap):
    """Exact transportation: pair-class -> eligible quarters, cap per quarter.
    Returns {pair: {q: n}} or None. Dinic on a tiny graph."""
    elig = {}
    for p, n in cnt_by_pair.items():
        i, j = p
        qs = tuple(q for q, S in enumerate(blocks) if i in S and j in S)
        if not qs:
            return None
        elig.setdefault(qs, []).append(p)
    classes = list(elig)
    C, Q = len(classes), len(blocks)
    S, Tk = 0, C + Q + 1
    cap_m = {}
    def add(u, v, c):
        cap_m[(u, v)] = cap_m.get((u, v), 0) + c
        cap_m.setdefault((v, u), 0)
    total = 0
    for ci, k in enumerate(classes):
        n = sum(cnt_by_pair[p] for p in elig[k])
        add(S, 1 + ci, n)
        total += n
        for q in k:
            add(1 + ci, 1 + C + q, n)
    for q in range(Q):
        add(1 + C + q, Tk, cap)
    from collections import deque
    adj = {}
    for (u, v) in cap_m:
        adj.setdefault(u, []).append(v)
    flow_tot = 0
    while True:
        lvl = {S: 0}
        dq = deque([S])
        while dq:
            u = dq.popleft()
            for v in adj.get(u, []):
                if v not in lvl and cap_m[(u, v)] > 0:
                    lvl[v] = lvl[u] + 1
                    dq.append(v)
        if Tk not in lvl:
            break
        it = {u: 0 for u in adj}
        def dfs(u, f):
            if u == Tk:
                return f
            while it[u] < len(adj[u]):
                v = adj[u][it[u]]
                if cap_m[(u, v)] > 0 and lvl.get(v, -1) == lvl[u] + 1:
                    d = dfs(v, min(f, cap_m[(u, v)]))
                    if d > 0:
                        cap_m[(u, v)] -= d
                        cap_m[(v, u)] += d
                        return d
                it[u] += 1
            return 0
        while True:
            f = dfs(S, 1 << 30)
            if f == 0:
                break
            flow_tot += f
    if flow_tot != total:
        return None
    out = {}
    for ci, k in enumerate(classes):
        got = {q: cap_m[(1 + C + q, 1 + ci)] for q in k
               if cap_m[(1 + C + q, 1 + ci)] > 0}
        pairs = elig[k]
        qiter = [(q, n) for q, n in got.items()]
        qi, left = 0, qiter[0][1] if qiter else 0
        for p in pairs:
            need = cnt_by_pair[p]
            out[p] = {}
            while need > 0:
                q, _ = qiter[qi]
                take = min(need, left)
                out[p][q] = out[p].get(q, 0) + take
                need -= take
                left -= take
                if left == 0 and qi + 1 < len(qiter):
                    qi += 1
                    left = qiter[qi][1]
    return out


def _route_and_balance(sel):
    """Host balancing: tokens (with top-2 expert pairs) -> 4 quarters of T
    tokens, each quarter covering its pairs with `slots` experts."""
    pair_of = [tuple(sorted(sel[t])) for t in range(NT)]
    cnt = {}
    toks_by_pair = {}
    for t, p in enumerate(pair_of):
        cnt[p] = cnt.get(p, 0) + 1
        toks_by_pair.setdefault(p, []).append(t)

    import itertools
    rng = np.random.RandomState(0)
    all5 = list(itertools.combinations(range(8), 5))

    def try_blocks(blocks):
        if not all(any(i in S and j in S for S in blocks)
                   for i in range(8) for j in range(i + 1, 8)):
            return None
        return _maxflow_assign(cnt, blocks, T)

    solution = None
    for trial in range(4000):
        idx = rng.choice(len(all5), 4, replace=True)
        blocks = [set(all5[i]) for i in idx]
        r = try_blocks(blocks)
        if r is not None:
            solution = (blocks, r, 5)
            break
    if solution is None:
        all6 = list(itertools.combinations(range(8), 6))
        for trial in range(4000):
            idx = rng.choice(len(all6), 4, replace=True)
            blocks = [set(all6[i]) for i in idx]
            r = try_blocks(blocks)
            if r is not None:
                solution = (blocks, r, 6)
                break
    if solution is None:
        blocks = [set(range(8))] * 4
        solution = (blocks, _maxflow_assign(cnt, blocks, T), 8)

    blocks, assign, slots = solution
    qtoks = [[] for _ in range(TQ)]
    for p, qmap in assign.items():
        toks = toks_by_pair[p]
        i = 0
        for q, n in qmap.items():
            qtoks[q].extend(toks[i:i + n])
            i += n
    perm = np.concatenate([np.array(sorted(qt), dtype=np.int64) for qt in qtoks])
    slot_experts = [sorted(blocks[q]) for q in range(TQ)]
    return perm, slot_experts, slots


def _pack_inputs(hidden_states, gate, W1, b1, W2, b2, A1, B1, A2, B2):
    hs = np.asarray(hidden_states, dtype=np.float64)
    x = hs.reshape(NT, D)
    logits = x @ np.asarray(gate, np.float64).T
    order = np.argsort(-logits, axis=1, kind="stable")
    sel = order[:, :2]                                   # [NT, 2]
    l12 = np.take_along_axis(logits, sel, axis=1)
    w1r = 1.0 / (1.0 + np.exp(-(l12[:, 0] - l12[:, 1])))
    wts = np.stack([w1r, 1.0 - w1r], axis=1)             # [NT, 2]

    perm, slot_experts, slots = _route_and_balance(sel)

    xT = np.ascontiguousarray(x[perm].T.astype(np.float32))    # [D, NT] permuted
    sel_p = sel[perm]
    wts_p = wts[perm]

    W1T = np.asarray(W1, np.float32).T                   # [D, H]
    w1p_full = np.ascontiguousarray(
        W1T.reshape(KD, 128, H // 128, 128).transpose(2, 1, 0, 3)
        .reshape(H // 128, 128, KD * 128)).astype(BF)
    W2T = np.asarray(W2, np.float32).T                   # [H, D]
    w2p_full = np.ascontiguousarray(
        W2T.reshape(H // 128, 128, MD, 128).transpose(2, 1, 0, 3)
        .reshape(MD, 128, (H // 128) * 128)).astype(BF)

    A1 = np.asarray(A1, np.float32)
    B1 = np.asarray(B1, np.float32)
    A2 = np.asarray(A2, np.float32)
    B2 = np.asarray(B2, np.float32)

    assert not np.asarray(b1).any() and not np.asarray(b2).any(), \
        "nonzero biases not supported by this build"

    per_q = []
    for q in range(TQ):
        ex = slot_experts[q]
        S = np.zeros((D, 256), np.float32)
        b1d_full = np.zeros((2, 128, H), np.float32)
        arr = np.zeros((H, 256), np.float32)
        b2sA = np.zeros((2, 128, D), np.float32)
        for si in range(slots):
            s, g = divmod(si, 4)
            base = s * 128 + 32 * g
            S[:, base:base + 16] = A1[ex[si]].T
            b1d_full[s, 32 * g:32 * g + 16, :] = SC * B1[ex[si]].T
            if si > 0:
                S[:, base + 16:base + 32] = A1[ex[si - 1]].T
                b1d_full[s, 32 * g + 16:32 * g + 32, :] = -SC * B1[ex[si - 1]].T
            arr[:, base:base + 16] = A2[ex[si]].T
            b2sA[s, 32 * g:32 * g + 16, :] = SC * B2[ex[si]].T
        a1s = np.ascontiguousarray(
            S.reshape(KD, 128, 256).transpose(1, 0, 2)
            .reshape(128, KD * 256)).astype(BF)
        a2s_full = np.ascontiguousarray(
            arr.reshape(H // 128, 128, 256).transpose(1, 0, 2)
            .reshape(128, (H // 128) * 256)).astype(BF)
        b2qA = np.zeros((128, MD * 2 * 128), np.float32)
        for m2 in range(MD):
            for s in range(2):
                b2qA[:, (m2 * 2 + s) * 128:(m2 * 2 + s + 1) * 128] = \
                    b2sA[s][:, m2 * 128:(m2 + 1) * 128]

        tq_sel = sel_p[q * T:(q + 1) * T]
        tq_wts = wts_p[q * T:(q + 1) * T]
        crow = np.zeros((slots, T), np.float64)
        for si in range(slots):
            m = (tq_sel == ex[si])
            crow[si] = (tq_wts * m).sum(axis=1)
        cbcA = np.ascontiguousarray(
            np.broadcast_to(crow.reshape(1, slots * T), (128, slots * T))
        ).astype(BF)
        czA = np.zeros((2, 128, T), np.float32)
        for si in range(slots):
            s, j = divmod(si, 4)
            czA[s, 32 * j:32 * j + 32, :] = crow[si]
        per_q.append((a1s, b1d_full.astype(BF), a2s_full, b2qA.astype(F8),
                      cbcA, czA.astype(BF)))

    in_maps = []
    for c in range(NCORES):
        tq, hh = divmod(c, HH)
        a1s, b1d_full, a2s_full, b2qA, cbcA, czA = per_q[tq]
        xc = xT[:, tq * T:(tq + 1) * T]
        xcp = np.ascontiguousarray(
            xc.reshape(KD, 128, T).transpose(1, 0, 2).reshape(128, KD * T))
        msl = slice(hh * MH, (hh + 1) * MH)
        in_maps.append({
            "xtb": xcp.astype(BF),
            "w1p": np.ascontiguousarray(w1p_full[msl]),
            "w2p": np.ascontiguousarray(
                w2p_full[:, :, hh * MH * 128:(hh + 1) * MH * 128]),
            "a1s": a1s,
            "b1d": np.ascontiguousarray(b1d_full[:, :, hh * HL:(hh + 1) * HL]),
            "a2s": np.ascontiguousarray(
                a2s_full[:, hh * MH * 256:(hh + 1) * MH * 256]),
            "b2q": b2qA,
            "cbc": cbcA,
            "cz": np.ascontiguousarray(
                czA.transpose(1, 0, 2).reshape(128, 2 * T)),
        })
    return in_maps, perm, slots


_NC_CACHE = {}


def get_nc(slots):
    if slots not in _NC_CACHE:
        _NC_CACHE[slots] = _build_bass(slots)
    return _NC_CACHE[slots]


def _unpack_outputs(results, perm):
    cols = []
    for tq in range(TQ):
        o = None
        for hh in range(HH):
            c = tq * HH + hh
            p = np.asarray(results[c]["outt"], np.float32)
            p = p.reshape(128, MD, T).transpose(1, 0, 2).reshape(D, T)
            o = p if o is None else o + p
        cols.append(o)
    outT = np.concatenate(cols, axis=1)                  # [D, NT] (permuted tokens)
    out = np.empty((NT, D), np.float32)
    out[perm] = outT.T
    return out.reshape(2, NT // 2, D)


def kernel(**inputs):
    in_maps, perm, slots = _pack_inputs(**inputs)
    nc = get_nc(slots)
    res = run_bass_kernel_spmd(nc, in_maps, core_ids=list(range(NCORES)))
    return _unpack_outputs(res.results, perm)
